# revision 19
# baseline (speedup 1.0000x reference)
"""RWKV WKV attention kernel for 8 Trainium2 NeuronCores.

Sharding: core i handles (batch b = i//2, time-half h = i%2), i.e. 1024 tokens
of one batch element. The WKV recurrence state is NOT exchanged between the
two halves: each core recomputes a 32-token warmup halo before its real
tokens. The per-step decay is e^{-w} with w = exp(time_decay) >= e^{-1}, so
the halo truncation error is ~8e-6 — far below f32 noise.

Per-core pipeline (all on-chip, layout [channels on partitions, time on free]):
  1. x window shipped partition-major and DMA'd as 4 fat-packet chunks on one
     queue (~315 GB/s); cur/prev are shifted views of the single window tile.
     dx = xc - xp is stored into the xv tile, consumed by the xk/xr mixes,
     then overwritten in place by the xv mix (one STT). Mix work is split
     DVE/ACT with xk produced first.
  2. k/v/r projections as bf16 matmuls, f32 PSUM (TensorE), emitted with a
     5-block skew (k[0..4] first) so TensorE never waits on the xv/xr mixes.
  3. ek = exp(k) (ACT; warmup columns zeroed via a -1e4 bias on h=0 cores),
     er = exp(-r) (ACT, same Exp table), P = ek*v (DVE)
  4. A/B linear recurrences via tensor_tensor_scan (DVE, f32 state)
  5. wkv*sigmoid(r) = num / (den*(1+er)); reciprocal in place on ACT
  6. z -> output projection matmul -> f32 out. The last block's epilogue is
     split in column halves so the output projection's first accumulation
     chain doesn't stall on z[15].
"""
import os
import sys

for _p in ("/opt/trn_rl_repo", "/root/.axon_site/_ro/trn_rl_repo"):
    if os.path.isdir(_p) and _p not in sys.path:
        sys.path.append(_p)

import numpy as np
import ml_dtypes

B, T, D = 4, 2048, 2048
H = T // 2          # tokens per core
L = 32              # warmup halo tokens
W = H + L           # scan window per core
W1 = W + 1          # window incl. one prev token
P = 128             # partitions
G = D // P          # channel blocks
N_CORES = 8
SKEW = 5            # k-projection blocks emitted ahead of the v/r stream
NPAR = 6            # per-channel params: [emw, eu, mk, mv, mr, warm_bias]

bf16 = ml_dtypes.bfloat16

_compat_installed = False
_built = None


def _install_compat():
    """Split the TileContext exit-drain's sem waits across single-wait nops
    (this walrus build rejects CTRL instructions with >1 sync wait)."""
    global _compat_installed
    if _compat_installed:
        return
    import concourse.mybir as mybir
    import concourse.tile as tile
    from concourse.vector_clock import ScopedClock

    def patched_drain_and_barrier(self, tick_clock, wait_clock):
        nop_inst = self.nc.sync.nop(nofuse=True, hint="drain_split")
        wait_clock.add_sem_waits(
            nop_inst.ins, ScopedClock({None: tick_clock.global_clock})
        )
        si = nop_inst.ins.sync_info
        if si and si.on_wait and len(si.on_wait) > 1:
            waits = list(si.on_wait)
            del si.on_wait[1:]
            for w in waits[1:]:
                extra = self.nc.sync.nop(nofuse=True, hint="drain_split2")
                esi = extra.ins.sync_info
                if esi is None:
                    extra.ins.sync_info = mybir.SyncInfo(on_wait=[w], on_update=[])
                else:
                    esi.on_wait.append(w)
        self.nc.sync.drain()
        self.nc.all_engine_barrier()
        popped = self.nc._tile_sem_poison_stack.pop()
        assert popped is self._sem_poison
        self.nc.clear_and_free_semaphores(list(self.sems.allocated().values()))
        self.nc.all_engine_barrier()

    tile.TileContext._drain_and_barrier = patched_drain_and_barrier
    _compat_installed = True


def _split_multi_waits(nc):
    """This walrus build allows at most ONE sync wait per instruction; hoist
    extra waits onto same-engine NoOps placed just before the instruction."""
    import concourse.mybir as mybir

    n_split = 0
    for fn in nc.m.functions:
        for blk in fn.blocks:
            new_insts = []
            for inst in blk.instructions:
                si = inst.sync_info
                if si is not None and si.on_wait and len(si.on_wait) > 1:
                    waits = list(si.on_wait)
                    for j, w in enumerate(waits[:-1]):
                        nop = mybir.InstNoOp(
                            name=f"{inst.name}-wsplit{j}",
                            engine=inst.engine,
                            ins=[],
                            outs=[],
                            sync_info=mybir.SyncInfo(on_wait=[w], on_update=[]),
                        )
                        new_insts.append(nop)
                    del si.on_wait[:-1]
                    n_split += 1
                new_insts.append(inst)
            blk.instructions = new_insts
    return n_split


def _act_reciprocal(nc, out, in_):
    """ACT-table reciprocal (bass blocks it by default over accuracy concerns;
    measured end-to-end error here is well within tolerance, and it keeps the
    division off the critical DVE engine)."""
    import concourse.mybir as mybir

    eng = nc.scalar
    inputs = [
        eng.lower_ap(in_),
        mybir.ImmediateValue(dtype=mybir.dt.float32, value=0.0),
        mybir.ImmediateValue(dtype=mybir.dt.float32, value=1.0),
        mybir.ImmediateValue(dtype=mybir.dt.float32, value=0.0),
    ]
    return eng.add_instruction(
        mybir.InstActivation(
            name=nc.get_next_instruction_name(),
            func=mybir.ActivationFunctionType.Reciprocal,
            ins=inputs,
            outs=[eng.lower_ap(out)],
        )
    )


def build_graph():
    """Build the SPMD Bass graph (identical on all 8 cores)."""
    _install_compat()
    import concourse.bass as bass
    import concourse.mybir as mybir
    import concourse.tile as tile
    from concourse.alu_op_type import AluOpType as Op

    F32 = mybir.dt.float32
    BF16 = mybir.dt.bfloat16
    ACTF = mybir.ActivationFunctionType

    nc = bass.Bass("TRN2", num_devices=N_CORES)

    # x window, partition-major: xin[p, g*W1 + t] = x_window[t, g*128 + p]
    xin_ext = nc.declare_dram_parameter("xin", [P, G * W1], BF16, isOutput=False)
    wk_ext = nc.declare_dram_parameter("wk", [G, P, D], BF16, isOutput=False)
    wv_ext = nc.declare_dram_parameter("wv", [G, P, D], BF16, isOutput=False)
    wr_ext = nc.declare_dram_parameter("wr", [G, P, D], BF16, isOutput=False)
    wo_ext = nc.declare_dram_parameter("wo", [G, P, D], BF16, isOutput=False)
    # params, partition-major: [P, G*NPAR]
    par_ext = nc.declare_dram_parameter("params", [P, G * NPAR], F32, isOutput=False)
    out_ext = nc.declare_dram_parameter("out", [D, H], BF16, isOutput=True)

    # k/v projections cover the warmup + real window (W cols);
    # r and the output projection cover only the real window (H cols).
    KV_CHUNKS = [(0, L), (L, 512), (L + 512, 512)]
    R_CHUNKS = [(0, 512), (512, 512)]

    with tile.TileContext(nc) as tc:
        with (
            tc.tile_pool(name="const", bufs=1) as constp,
            tc.tile_pool(name="xz", bufs=1) as xzp,
            tc.tile_pool(name="mix", bufs=1) as mixp,
            tc.tile_pool(name="scr", bufs=1) as scrp,
            tc.tile_pool(name="wt", bufs=1) as wtp,
            tc.tile_pool(name="ep", bufs=1) as epp,
            tc.tile_pool(name="ob", bufs=1) as obp,
            tc.tile_pool(name="ps", bufs=1, space="PSUM") as psp,
        ):
            # ---- params: plain 2D fat-packet DMA on the scalar queue ----
            par = constp.tile([P, G * NPAR], F32, tag="par", name="par")
            nc.scalar.dma_start(par[:], par_ext[:, :])

            def pp(g, j):  # per-partition scalar AP for block g, param j
                return par[:, g * NPAR + j : g * NPAR + j + 1]

            # ---- x window: 4 fat chunks on the sync queue ----
            xw = xzp.tile([P, G * W1], BF16, tag="xz", name="xw")
            for q in range(4):
                cs = slice(q * 4 * W1, (q + 1) * 4 * W1)
                nc.sync.dma_start(xw[:, cs], xin_ext[:, cs])

            def xcur(g):
                return xw[:, g * W1 + 1 : g * W1 + 1 + W]

            def xprev(g):
                return xw[:, g * W1 : g * W1 + W]

            # ---- mixes: dx lands in the xv tile, is read by the xk/xr
            # mixes, then overwritten in place by the xv mix ----
            xk = mixp.tile([P, G * W], BF16, tag="xk", name="xk")
            xv = mixp.tile([P, G * W], BF16, tag="xv", name="xv")
            xr = mixp.tile([P, G * H], BF16, tag="xr", name="xr")

            def dx(g):
                return xv[:, g * W : (g + 1) * W]

            # pass 1: dx + xk (the first-projection critical path)
            for g in range(G):
                nc.vector.tensor_tensor(dx(g), xcur(g), xprev(g), Op.subtract)
                tmp = scrp.tile([P, W], BF16, tag="tmp", name=f"tk{g}", bufs=2)
                nc.scalar.activation(tmp[:], dx(g), ACTF.Copy, scale=pp(g, 2))
                nc.vector.tensor_tensor(
                    xk[:, g * W : (g + 1) * W], tmp[:], xprev(g), Op.add
                )

            # pass 2: xr via ACT-scale + add; xv via one in-place STT
            for g in range(G):
                tmpr = scrp.tile([P, H], BF16, tag="tmp", name=f"tr{g}", bufs=2)
                nc.scalar.activation(tmpr[:], dx(g)[:, L:W], ACTF.Copy, scale=pp(g, 4))
                # xv = (dx * mv) + xp, overwriting dx (reads dx before write,
                # elementwise in place)
                nc.vector.scalar_tensor_tensor(
                    xv[:, g * W : (g + 1) * W], dx(g), pp(g, 3), xprev(g),
                    Op.mult, Op.add,
                )
                nc.vector.tensor_tensor(
                    xr[:, g * H : (g + 1) * H], tmpr[:],
                    xw[:, g * W1 + L : g * W1 + L + H], Op.add,
                )

            # ---- weight tiles + matmul emission with skew ----
            wk_t, wv_t, wr_t = {}, {}, {}
            ek, pv_ps, pr_ps = {}, {}, {}

            def load_w(pool_tag, ext, m, eng=None, bufs=2):
                t = wtp.tile([P, D], BF16, tag=pool_tag, name=f"{pool_tag}{m}",
                             bufs=bufs)
                (eng or nc.sync).dma_start(t[:], ext[m])
                return t

            def emit_k(m):
                wt = wk_t.pop(m)
                pks = []
                for ci, (c0, cw) in enumerate(KV_CHUNKS):
                    pks.append(
                        psp.tile([P, cw], F32, tag=f"pk{ci}", name=f"pk{ci}_{m}", bufs=2)
                    )
                for g in range(G):
                    lhs = wt[:, g * P : (g + 1) * P]
                    for ci, (c0, cw) in enumerate(KV_CHUNKS):
                        nc.tensor.matmul(
                            pks[ci][:], lhs, xk[:, g * W + c0 : g * W + c0 + cw],
                            start=(g == 0), stop=(g == G - 1),
                        )
                # ek = exp(k); warmup chunk gets the zeroing bias
                ekt = epp.tile([P, W], BF16, tag="ek", name=f"ek{m}", bufs=SKEW)
                for ci, (c0, cw) in enumerate(KV_CHUNKS):
                    bias = pp(m, 5) if ci == 0 else 0.0
                    nc.scalar.activation(
                        ekt[:, c0 : c0 + cw], pks[ci][:], ACTF.Exp, bias=bias
                    )
                ek[m] = ekt

            def emit_v(m):
                wt = wv_t.pop(m)
                pvs = []
                for ci, (c0, cw) in enumerate(KV_CHUNKS):
                    pvs.append(
                        psp.tile([P, cw], F32, tag=f"pk{ci}", name=f"pv{ci}_{m}", bufs=2)
                    )
                for g in range(G):
                    lhs = wt[:, g * P : (g + 1) * P]
                    for ci, (c0, cw) in enumerate(KV_CHUNKS):
                        nc.tensor.matmul(
                            pvs[ci][:], lhs, xv[:, g * W + c0 : g * W + c0 + cw],
                            start=(g == 0), stop=(g == G - 1),
                        )
                pv_ps[m] = pvs

            def emit_r(m):
                wt = wr_t.pop(m)
                prs = []
                for ci, (c0, cw) in enumerate(R_CHUNKS):
                    prs.append(
                        psp.tile([P, cw], F32, tag=f"pr{ci}", name=f"pr{ci}_{m}", bufs=1)
                    )
                for g in range(G):
                    lhs = wt[:, g * P : (g + 1) * P]
                    for ci, (c0, cw) in enumerate(R_CHUNKS):
                        nc.tensor.matmul(
                            prs[ci][:], lhs, xr[:, g * H + c0 : g * H + c0 + cw],
                            start=(g == 0), stop=(g == G - 1),
                        )
                pr_ps[m] = prs

            zall = None
            z15 = [None, None]

            def emit_epi(m):
                nonlocal zall
                ekt = ek.pop(m)
                pvs = pv_ps.pop(m)
                prs = pr_ps.pop(m)
                # er = exp(-r): same ACT table as exp(k); sigmoid is folded
                # into the denominator as den * (1 + er)
                er = epp.tile([P, H], BF16, tag="er", name=f"er{m}")
                for ci, (c0, cw) in enumerate(R_CHUNKS):
                    nc.scalar.activation(
                        er[:, c0 : c0 + cw], prs[ci][:], ACTF.Exp, scale=-1.0
                    )
                pt = epp.tile([P, W], BF16, tag="pt", name=f"pt{m}")
                for ci, (c0, cw) in enumerate(KV_CHUNKS):
                    nc.vector.tensor_tensor(
                        pt[:, c0 : c0 + cw], ekt[:, c0 : c0 + cw], pvs[ci][:],
                        Op.mult,
                    )
                dec = pp(m, 0).broadcast_to([P, W])
                ab = epp.tile([P, W], F32, tag="ab", name=f"ab{m}")
                nc.vector.tensor_tensor_scan(ab[:], dec, pt[:], 0.0, Op.mult, Op.add)
                bb = epp.tile([P, W], F32, tag="bb", name=f"bb{m}")
                nc.vector.tensor_tensor_scan(bb[:], dec, ekt[:], 0.0, Op.mult, Op.add)

                num = epp.tile([P, H], F32, tag="num", name=f"num{m}")
                den = epp.tile([P, H], F32, tag="den", name=f"den{m}")
                if m < G - 1:
                    if zall is None:
                        zall = xzp.tile([P, (G - 1) * H], BF16, tag="xz", name="zall")
                    halves = [(0, H, zall[:, m * H : (m + 1) * H])]
                else:
                    z15[0] = xzp.tile([P, 512], BF16, tag="z15a", name="z15a")
                    z15[1] = xzp.tile([P, 512], BF16, tag="z15b", name="z15b")
                    halves = [(0, 512, z15[0][:]), (512, 512, z15[1][:])]
                for c0, cw, zdst in halves:
                    nsl = num[:, c0 : c0 + cw]
                    dsl = den[:, c0 : c0 + cw]
                    nc.vector.scalar_tensor_tensor(
                        nsl, pt[:, L + c0 : L + c0 + cw], pp(m, 1),
                        ab[:, L - 1 + c0 : L - 1 + c0 + cw], Op.mult, Op.add,
                    )
                    nc.vector.scalar_tensor_tensor(
                        dsl, ekt[:, L + c0 : L + c0 + cw], pp(m, 1),
                        bb[:, L - 1 + c0 : L - 1 + c0 + cw], Op.mult, Op.add,
                    )
                    # den *= (1 + er)  — folds sigmoid(r) into the denominator
                    nc.vector.scalar_tensor_tensor(
                        dsl, er[:, c0 : c0 + cw], 1.0, dsl, Op.add, Op.mult,
                    )
                    _act_reciprocal(nc, dsl, dsl)
                    nc.vector.tensor_tensor(zdst, nsl, dsl, Op.mult)

            # prefetch weights; the first two per projection ride the scalar
            # queue so they aren't serialized behind the x window
            wk_t[0] = load_w("wk", wk_ext, 0, eng=nc.scalar, bufs=3)
            wk_t[1] = load_w("wk", wk_ext, 1, eng=nc.scalar, bufs=3)
            wk_t[2] = load_w("wk", wk_ext, 2, eng=nc.scalar, bufs=3)
            for m in range(2):
                wv_t[m] = load_w("wv", wv_ext, m, eng=nc.scalar)
                wr_t[m] = load_w("wr", wr_ext, m, eng=nc.scalar)
            for m in range(SKEW):
                emit_k(m)
                if m + 3 < G:
                    wk_t[m + 3] = load_w("wk", wk_ext, m + 3, bufs=3)

            for m in range(G):
                emit_v(m)
                emit_r(m)
                if m + SKEW < G:
                    emit_k(m + SKEW)
                    if m + SKEW + 3 < G:
                        wk_t[m + SKEW + 3] = load_w("wk", wk_ext, m + SKEW + 3,
                                                    bufs=3)
                if m + 2 < G:
                    wv_t[m + 2] = load_w("wv", wv_ext, m + 2)
                    wr_t[m + 2] = load_w("wr", wr_ext, m + 2)
                emit_epi(m)

            # ---- output projection (z15 is read last in each chain) ----
            wo_t = {0: load_w("wk", wo_ext, 0, bufs=3),
                    1: load_w("wk", wo_ext, 1, bufs=3)}
            for m in range(G):
                wt = wo_t.pop(m)
                if m + 2 < G:
                    wo_t[m + 2] = load_w("wk", wo_ext, m + 2, bufs=3)
                for ci, (c0, cw) in enumerate(R_CHUNKS):
                    ps = psp.tile(
                        [P, cw], F32, tag=f"pk{ci + 1}", name=f"po{ci}_{m}", bufs=2
                    )
                    for g in range(G):
                        lhs = wt[:, g * P : (g + 1) * P]
                        if g < G - 1:
                            rhs = zall[:, g * H + c0 : g * H + c0 + cw]
                        else:
                            rhs = z15[ci][:]
                        nc.tensor.matmul(
                            ps[:], lhs, rhs, start=(g == 0), stop=(g == G - 1)
                        )
                    osb = obp.tile([P, cw], BF16, tag="osb", name=f"osb{m}_{ci}", bufs=2)
                    # the last block's copy+DMA are split so the final
                    # drain waits on a 256-col pipelined tail, not a full
                    # 512-col serial chain
                    parts = 2 if m == G - 1 else 1
                    pw = cw // parts
                    for j in range(parts):
                        nc.scalar.activation(
                            osb[:, j * pw : (j + 1) * pw], ps[:, j * pw : (j + 1) * pw],
                            ACTF.Copy,
                        )
                        nc.scalar.dma_start(
                            out_ext[m * P : (m + 1) * P, c0 + j * pw : c0 + (j + 1) * pw],
                            osb[:, j * pw : (j + 1) * pw],
                        )

    _split_multi_waits(nc)
    return nc


def _tile_weight(wt):
    """(D, D) f32 weight -> (G, P, D) bf16 lhsT tiles: [m][dp][g*128+ef]."""
    wT = np.ascontiguousarray(wt.T).astype(np.float32)
    t = wT.reshape(G, P, G, P).transpose(2, 1, 0, 3).reshape(G, P, D)
    return np.ascontiguousarray(t).astype(bf16)


def prepare_inputs(x, time_decay, time_first, time_mix_k, time_mix_v,
                   time_mix_r, Wk, Wv, Wr, Wo):
    x = np.asarray(x, np.float32)
    emw = np.exp(-np.exp(np.asarray(time_decay, np.float64))).astype(np.float32)
    eu = np.exp(np.asarray(time_first, np.float64)).astype(np.float32)
    mk = np.asarray(time_mix_k, np.float32).reshape(D)
    mv = np.asarray(time_mix_v, np.float32).reshape(D)
    mr = np.asarray(time_mix_r, np.float32).reshape(D)

    wk_t = _tile_weight(np.asarray(Wk))
    wv_t = _tile_weight(np.asarray(Wv))
    wr_t = _tile_weight(np.asarray(Wr))
    wo_t = _tile_weight(np.asarray(Wo))

    in_maps = []
    for core in range(N_CORES):
        b, h = divmod(core, 2)
        t0 = h * H
        xb = np.zeros((T + L + 1, D), np.float32)
        xb[L + 1 :] = x[b]
        # window rows [t0 .. t0+W] in padded coords = tokens [t0-L-1 .. t0+H-1]
        win = xb[t0 : t0 + W1]                           # (W1, D)
        # partition-major: xin[p, g*W1 + t] = win[t, g*128 + p]
        xin = np.ascontiguousarray(
            win.T.reshape(G, P, W1).transpose(1, 0, 2).reshape(P, G * W1)
        ).astype(bf16)
        warm_bias = np.full(D, 0.0 if h == 1 else -10000.0, np.float32)
        params = np.stack([emw, eu, mk, mv, mr, warm_bias], axis=1)  # (D, NPAR)
        params = np.ascontiguousarray(
            params.reshape(G, P, NPAR).transpose(1, 0, 2).reshape(P, G * NPAR)
        ).astype(np.float32)
        in_maps.append({
            "xin": xin,
            "wk": wk_t, "wv": wv_t, "wr": wr_t, "wo": wo_t,
            "params": params,
        })
    return in_maps


def get_graph():
    global _built
    if _built is None:
        _built = build_graph()
    return _built


def kernel(**inputs) -> np.ndarray:
    from concourse.bass_utils import run_bass_kernel_spmd

    nc = get_graph()
    in_maps = prepare_inputs(**inputs)
    res = run_bass_kernel_spmd(nc, in_maps, list(range(N_CORES)))
    out = np.empty((B, T, D), np.float32)
    for core in range(N_CORES):
        b, h = divmod(core, 2)
        out[b, h * H : (h + 1) * H, :] = (
            res.results[core]["out"].astype(np.float32).T
        )
    return out


# revision 20
# speedup vs baseline: 1.0044x; 1.0044x over previous
"""RWKV WKV attention kernel for 8 Trainium2 NeuronCores.

Sharding: core i handles (batch b = i//2, time-half h = i%2), i.e. 1024 tokens
of one batch element. The WKV recurrence state is NOT exchanged between the
two halves: each core recomputes a 32-token warmup halo before its real
tokens. The per-step decay is e^{-w} with w = exp(time_decay) >= e^{-1}, so
the halo truncation error is ~8e-6 — far below f32 noise.

Per-core pipeline (all on-chip, layout [channels on partitions, time on free]):
  1. x window shipped partition-major and DMA'd as 4 fat-packet chunks on one
     queue (~315 GB/s); cur/prev are shifted views of the single window tile.
     dx = xc - xp is stored into the xv tile, consumed by the xk/xr mixes,
     then overwritten in place by the xv mix (one STT). Mix work is split
     DVE/ACT with xk produced first.
  2. k/v/r projections as bf16 matmuls, f32 PSUM (TensorE), emitted with a
     5-block skew (k[0..4] first) so TensorE never waits on the xv/xr mixes.
  3. ek = exp(k) (ACT; warmup columns zeroed via a -1e4 bias on h=0 cores),
     er = exp(-r) (ACT, same Exp table), P = ek*v (DVE)
  4. A/B linear recurrences via tensor_tensor_scan (DVE, f32 state)
  5. wkv*sigmoid(r) = num / (den*(1+er)); reciprocal in place on ACT
  6. z -> output projection matmul -> f32 out. The last block's epilogue is
     split in column halves so the output projection's first accumulation
     chain doesn't stall on z[15].
"""
import os
import sys

for _p in ("/opt/trn_rl_repo", "/root/.axon_site/_ro/trn_rl_repo"):
    if os.path.isdir(_p) and _p not in sys.path:
        sys.path.append(_p)

import numpy as np
import ml_dtypes

B, T, D = 4, 2048, 2048
H = T // 2          # tokens per core
L = 32              # warmup halo tokens
W = H + L           # scan window per core
W1 = W + 1          # window incl. one prev token
P = 128             # partitions
G = D // P          # channel blocks
N_CORES = 8
SKEW = 5            # k-projection blocks emitted ahead of the v/r stream
NPAR = 6            # per-channel params: [emw, eu, mk, mv, mr, warm_bias]

bf16 = ml_dtypes.bfloat16

_compat_installed = False
_built = None


def _install_compat():
    """Split the TileContext exit-drain's sem waits across single-wait nops
    (this walrus build rejects CTRL instructions with >1 sync wait)."""
    global _compat_installed
    if _compat_installed:
        return
    import concourse.mybir as mybir
    import concourse.tile as tile
    from concourse.vector_clock import ScopedClock

    def patched_drain_and_barrier(self, tick_clock, wait_clock):
        nop_inst = self.nc.sync.nop(nofuse=True, hint="drain_split")
        wait_clock.add_sem_waits(
            nop_inst.ins, ScopedClock({None: tick_clock.global_clock})
        )
        si = nop_inst.ins.sync_info
        if si and si.on_wait and len(si.on_wait) > 1:
            waits = list(si.on_wait)
            del si.on_wait[1:]
            for w in waits[1:]:
                extra = self.nc.sync.nop(nofuse=True, hint="drain_split2")
                esi = extra.ins.sync_info
                if esi is None:
                    extra.ins.sync_info = mybir.SyncInfo(on_wait=[w], on_update=[])
                else:
                    esi.on_wait.append(w)
        self.nc.sync.drain()
        self.nc.all_engine_barrier()
        popped = self.nc._tile_sem_poison_stack.pop()
        assert popped is self._sem_poison
        self.nc.clear_and_free_semaphores(list(self.sems.allocated().values()))
        self.nc.all_engine_barrier()

    tile.TileContext._drain_and_barrier = patched_drain_and_barrier
    _compat_installed = True


def _split_multi_waits(nc):
    """This walrus build allows at most ONE sync wait per instruction; hoist
    extra waits onto same-engine NoOps placed just before the instruction."""
    import concourse.mybir as mybir

    n_split = 0
    for fn in nc.m.functions:
        for blk in fn.blocks:
            new_insts = []
            for inst in blk.instructions:
                si = inst.sync_info
                if si is not None and si.on_wait and len(si.on_wait) > 1:
                    waits = list(si.on_wait)
                    for j, w in enumerate(waits[:-1]):
                        nop = mybir.InstNoOp(
                            name=f"{inst.name}-wsplit{j}",
                            engine=inst.engine,
                            ins=[],
                            outs=[],
                            sync_info=mybir.SyncInfo(on_wait=[w], on_update=[]),
                        )
                        new_insts.append(nop)
                    del si.on_wait[:-1]
                    n_split += 1
                new_insts.append(inst)
            blk.instructions = new_insts
    return n_split


def _act_reciprocal(nc, out, in_):
    """ACT-table reciprocal (bass blocks it by default over accuracy concerns;
    measured end-to-end error here is well within tolerance, and it keeps the
    division off the critical DVE engine)."""
    import concourse.mybir as mybir

    eng = nc.scalar
    inputs = [
        eng.lower_ap(in_),
        mybir.ImmediateValue(dtype=mybir.dt.float32, value=0.0),
        mybir.ImmediateValue(dtype=mybir.dt.float32, value=1.0),
        mybir.ImmediateValue(dtype=mybir.dt.float32, value=0.0),
    ]
    return eng.add_instruction(
        mybir.InstActivation(
            name=nc.get_next_instruction_name(),
            func=mybir.ActivationFunctionType.Reciprocal,
            ins=inputs,
            outs=[eng.lower_ap(out)],
        )
    )


def build_graph():
    """Build the SPMD Bass graph (identical on all 8 cores)."""
    _install_compat()
    import concourse.bass as bass
    import concourse.mybir as mybir
    import concourse.tile as tile
    from concourse.alu_op_type import AluOpType as Op

    F32 = mybir.dt.float32
    BF16 = mybir.dt.bfloat16
    ACTF = mybir.ActivationFunctionType

    nc = bass.Bass("TRN2", num_devices=N_CORES)

    # x window, partition-major: xin[p, g*W1 + t] = x_window[t, g*128 + p]
    xin_ext = nc.declare_dram_parameter("xin", [P, G * W1], BF16, isOutput=False)
    wk_ext = nc.declare_dram_parameter("wk", [G, P, D], BF16, isOutput=False)
    wv_ext = nc.declare_dram_parameter("wv", [G, P, D], BF16, isOutput=False)
    wr_ext = nc.declare_dram_parameter("wr", [G, P, D], BF16, isOutput=False)
    wo_ext = nc.declare_dram_parameter("wo", [G, P, D], BF16, isOutput=False)
    # params, partition-major: [P, G*NPAR]
    par_ext = nc.declare_dram_parameter("params", [P, G * NPAR], F32, isOutput=False)
    out_ext = nc.declare_dram_parameter("out", [D, H], F32, isOutput=True)

    # k/v projections cover the warmup + real window (W cols);
    # r and the output projection cover only the real window (H cols).
    KV_CHUNKS = [(0, L), (L, 512), (L + 512, 512)]
    R_CHUNKS = [(0, 512), (512, 512)]

    with tile.TileContext(nc) as tc:
        with (
            tc.tile_pool(name="const", bufs=1) as constp,
            tc.tile_pool(name="xz", bufs=1) as xzp,
            tc.tile_pool(name="mix", bufs=1) as mixp,
            tc.tile_pool(name="scr", bufs=1) as scrp,
            tc.tile_pool(name="wt", bufs=1) as wtp,
            tc.tile_pool(name="ep", bufs=1) as epp,
            tc.tile_pool(name="ob", bufs=1) as obp,
            tc.tile_pool(name="ps", bufs=1, space="PSUM") as psp,
        ):
            # ---- params: plain 2D fat-packet DMA on the scalar queue ----
            par = constp.tile([P, G * NPAR], F32, tag="par", name="par")
            nc.scalar.dma_start(par[:], par_ext[:, :])

            def pp(g, j):  # per-partition scalar AP for block g, param j
                return par[:, g * NPAR + j : g * NPAR + j + 1]

            # ---- x window: 4 fat chunks on the sync queue ----
            xw = xzp.tile([P, G * W1], BF16, tag="xz", name="xw")
            for q in range(4):
                cs = slice(q * 4 * W1, (q + 1) * 4 * W1)
                nc.sync.dma_start(xw[:, cs], xin_ext[:, cs])

            def xcur(g):
                return xw[:, g * W1 + 1 : g * W1 + 1 + W]

            def xprev(g):
                return xw[:, g * W1 : g * W1 + W]

            # ---- mixes: dx lands in the xv tile, is read by the xk/xr
            # mixes, then overwritten in place by the xv mix ----
            xk = mixp.tile([P, G * W], BF16, tag="xk", name="xk")
            xv = mixp.tile([P, G * W], BF16, tag="xv", name="xv")
            xr = mixp.tile([P, G * H], BF16, tag="xr", name="xr")

            def dx(g):
                return xv[:, g * W : (g + 1) * W]

            # pass 1: dx + xk (the first-projection critical path)
            for g in range(G):
                nc.vector.tensor_tensor(dx(g), xcur(g), xprev(g), Op.subtract)
                tmp = scrp.tile([P, W], BF16, tag="tmp", name=f"tk{g}", bufs=2)
                nc.scalar.activation(tmp[:], dx(g), ACTF.Copy, scale=pp(g, 2))
                nc.vector.tensor_tensor(
                    xk[:, g * W : (g + 1) * W], tmp[:], xprev(g), Op.add
                )

            # pass 2: xr via ACT-scale + add; xv via one in-place STT
            for g in range(G):
                tmpr = scrp.tile([P, H], BF16, tag="tmp", name=f"tr{g}", bufs=2)
                nc.scalar.activation(tmpr[:], dx(g)[:, L:W], ACTF.Copy, scale=pp(g, 4))
                # xv = (dx * mv) + xp, overwriting dx (reads dx before write,
                # elementwise in place)
                nc.vector.scalar_tensor_tensor(
                    xv[:, g * W : (g + 1) * W], dx(g), pp(g, 3), xprev(g),
                    Op.mult, Op.add,
                )
                nc.vector.tensor_tensor(
                    xr[:, g * H : (g + 1) * H], tmpr[:],
                    xw[:, g * W1 + L : g * W1 + L + H], Op.add,
                )

            # ---- weight tiles + matmul emission with skew ----
            wk_t, wv_t, wr_t = {}, {}, {}
            ek, pv_ps, pr_ps = {}, {}, {}

            def load_w(pool_tag, ext, m, eng=None, bufs=2):
                t = wtp.tile([P, D], BF16, tag=pool_tag, name=f"{pool_tag}{m}",
                             bufs=bufs)
                (eng or nc.sync).dma_start(t[:], ext[m])
                return t

            def emit_k(m):
                wt = wk_t.pop(m)
                pks = []
                for ci, (c0, cw) in enumerate(KV_CHUNKS):
                    pks.append(
                        psp.tile([P, cw], F32, tag=f"pk{ci}", name=f"pk{ci}_{m}", bufs=2)
                    )
                for g in range(G):
                    lhs = wt[:, g * P : (g + 1) * P]
                    for ci, (c0, cw) in enumerate(KV_CHUNKS):
                        nc.tensor.matmul(
                            pks[ci][:], lhs, xk[:, g * W + c0 : g * W + c0 + cw],
                            start=(g == 0), stop=(g == G - 1),
                        )
                # ek = exp(k); warmup chunk gets the zeroing bias
                ekt = epp.tile([P, W], BF16, tag="ek", name=f"ek{m}", bufs=SKEW)
                for ci, (c0, cw) in enumerate(KV_CHUNKS):
                    bias = pp(m, 5) if ci == 0 else 0.0
                    nc.scalar.activation(
                        ekt[:, c0 : c0 + cw], pks[ci][:], ACTF.Exp, bias=bias
                    )
                ek[m] = ekt

            def emit_v(m):
                wt = wv_t.pop(m)
                pvs = []
                for ci, (c0, cw) in enumerate(KV_CHUNKS):
                    pvs.append(
                        psp.tile([P, cw], F32, tag=f"pk{ci}", name=f"pv{ci}_{m}", bufs=2)
                    )
                for g in range(G):
                    lhs = wt[:, g * P : (g + 1) * P]
                    for ci, (c0, cw) in enumerate(KV_CHUNKS):
                        nc.tensor.matmul(
                            pvs[ci][:], lhs, xv[:, g * W + c0 : g * W + c0 + cw],
                            start=(g == 0), stop=(g == G - 1),
                        )
                pv_ps[m] = pvs

            def emit_r(m):
                wt = wr_t.pop(m)
                prs = []
                for ci, (c0, cw) in enumerate(R_CHUNKS):
                    prs.append(
                        psp.tile([P, cw], F32, tag=f"pr{ci}", name=f"pr{ci}_{m}", bufs=1)
                    )
                for g in range(G):
                    lhs = wt[:, g * P : (g + 1) * P]
                    for ci, (c0, cw) in enumerate(R_CHUNKS):
                        nc.tensor.matmul(
                            prs[ci][:], lhs, xr[:, g * H + c0 : g * H + c0 + cw],
                            start=(g == 0), stop=(g == G - 1),
                        )
                pr_ps[m] = prs

            zall = None
            z15 = [None, None]

            def emit_epi(m):
                nonlocal zall
                ekt = ek.pop(m)
                pvs = pv_ps.pop(m)
                prs = pr_ps.pop(m)
                # er = exp(-r): same ACT table as exp(k); sigmoid is folded
                # into the denominator as den * (1 + er)
                er = epp.tile([P, H], BF16, tag="er", name=f"er{m}")
                for ci, (c0, cw) in enumerate(R_CHUNKS):
                    nc.scalar.activation(
                        er[:, c0 : c0 + cw], prs[ci][:], ACTF.Exp, scale=-1.0
                    )
                pt = epp.tile([P, W], BF16, tag="pt", name=f"pt{m}")
                for ci, (c0, cw) in enumerate(KV_CHUNKS):
                    nc.vector.tensor_tensor(
                        pt[:, c0 : c0 + cw], ekt[:, c0 : c0 + cw], pvs[ci][:],
                        Op.mult,
                    )
                dec = pp(m, 0).broadcast_to([P, W])
                ab = epp.tile([P, W], F32, tag="ab", name=f"ab{m}")
                nc.vector.tensor_tensor_scan(ab[:], dec, pt[:], 0.0, Op.mult, Op.add)
                bb = epp.tile([P, W], F32, tag="bb", name=f"bb{m}")
                nc.vector.tensor_tensor_scan(bb[:], dec, ekt[:], 0.0, Op.mult, Op.add)

                num = epp.tile([P, H], F32, tag="num", name=f"num{m}")
                den = epp.tile([P, H], F32, tag="den", name=f"den{m}")
                if m < G - 1:
                    if zall is None:
                        zall = xzp.tile([P, (G - 1) * H], BF16, tag="xz", name="zall")
                    halves = [(0, H, zall[:, m * H : (m + 1) * H])]
                else:
                    z15[0] = xzp.tile([P, 512], BF16, tag="z15a", name="z15a")
                    z15[1] = xzp.tile([P, 512], BF16, tag="z15b", name="z15b")
                    halves = [(0, 512, z15[0][:]), (512, 512, z15[1][:])]
                for c0, cw, zdst in halves:
                    nsl = num[:, c0 : c0 + cw]
                    dsl = den[:, c0 : c0 + cw]
                    nc.vector.scalar_tensor_tensor(
                        nsl, pt[:, L + c0 : L + c0 + cw], pp(m, 1),
                        ab[:, L - 1 + c0 : L - 1 + c0 + cw], Op.mult, Op.add,
                    )
                    nc.vector.scalar_tensor_tensor(
                        dsl, ekt[:, L + c0 : L + c0 + cw], pp(m, 1),
                        bb[:, L - 1 + c0 : L - 1 + c0 + cw], Op.mult, Op.add,
                    )
                    # den *= (1 + er)  — folds sigmoid(r) into the denominator
                    nc.vector.scalar_tensor_tensor(
                        dsl, er[:, c0 : c0 + cw], 1.0, dsl, Op.add, Op.mult,
                    )
                    _act_reciprocal(nc, dsl, dsl)
                    nc.vector.tensor_tensor(zdst, nsl, dsl, Op.mult)

            # prefetch weights; the first two per projection ride the scalar
            # queue so they aren't serialized behind the x window
            wk_t[0] = load_w("wk", wk_ext, 0, eng=nc.scalar, bufs=3)
            wk_t[1] = load_w("wk", wk_ext, 1, eng=nc.scalar, bufs=3)
            wk_t[2] = load_w("wk", wk_ext, 2, eng=nc.scalar, bufs=3)
            for m in range(2):
                wv_t[m] = load_w("wv", wv_ext, m, eng=nc.scalar)
                wr_t[m] = load_w("wr", wr_ext, m, eng=nc.scalar)
            for m in range(SKEW):
                emit_k(m)
                if m + 3 < G:
                    wk_t[m + 3] = load_w("wk", wk_ext, m + 3, bufs=3)

            for m in range(G):
                emit_v(m)
                emit_r(m)
                if m + SKEW < G:
                    emit_k(m + SKEW)
                    if m + SKEW + 3 < G:
                        wk_t[m + SKEW + 3] = load_w("wk", wk_ext, m + SKEW + 3,
                                                    bufs=3)
                if m + 2 < G:
                    wv_t[m + 2] = load_w("wv", wv_ext, m + 2)
                    wr_t[m + 2] = load_w("wr", wr_ext, m + 2)
                emit_epi(m)

            # ---- output projection (z15 is read last in each chain) ----
            wo_t = {0: load_w("wk", wo_ext, 0, bufs=3),
                    1: load_w("wk", wo_ext, 1, bufs=3)}
            for m in range(G):
                wt = wo_t.pop(m)
                if m + 2 < G:
                    wo_t[m + 2] = load_w("wk", wo_ext, m + 2, bufs=3)
                for ci, (c0, cw) in enumerate(R_CHUNKS):
                    ps = psp.tile(
                        [P, cw], F32, tag=f"pk{ci + 1}", name=f"po{ci}_{m}", bufs=2
                    )
                    for g in range(G):
                        lhs = wt[:, g * P : (g + 1) * P]
                        if g < G - 1:
                            rhs = zall[:, g * H + c0 : g * H + c0 + cw]
                        else:
                            rhs = z15[ci][:]
                        nc.tensor.matmul(
                            ps[:], lhs, rhs, start=(g == 0), stop=(g == G - 1)
                        )
                    osb = obp.tile([P, cw], F32, tag="osb", name=f"osb{m}_{ci}", bufs=2)
                    nc.scalar.activation(osb[:], ps[:], ACTF.Copy)
                    nc.scalar.dma_start(
                        out_ext[m * P : (m + 1) * P, c0 : c0 + cw], osb[:]
                    )

    _split_multi_waits(nc)
    return nc


def _tile_weight(wt):
    """(D, D) f32 weight -> (G, P, D) bf16 lhsT tiles: [m][dp][g*128+ef]."""
    wT = np.ascontiguousarray(wt.T).astype(np.float32)
    t = wT.reshape(G, P, G, P).transpose(2, 1, 0, 3).reshape(G, P, D)
    return np.ascontiguousarray(t).astype(bf16)


def prepare_inputs(x, time_decay, time_first, time_mix_k, time_mix_v,
                   time_mix_r, Wk, Wv, Wr, Wo):
    x = np.asarray(x, np.float32)
    emw = np.exp(-np.exp(np.asarray(time_decay, np.float64))).astype(np.float32)
    eu = np.exp(np.asarray(time_first, np.float64)).astype(np.float32)
    mk = np.asarray(time_mix_k, np.float32).reshape(D)
    mv = np.asarray(time_mix_v, np.float32).reshape(D)
    mr = np.asarray(time_mix_r, np.float32).reshape(D)

    wk_t = _tile_weight(np.asarray(Wk))
    wv_t = _tile_weight(np.asarray(Wv))
    wr_t = _tile_weight(np.asarray(Wr))
    wo_t = _tile_weight(np.asarray(Wo))

    in_maps = []
    for core in range(N_CORES):
        b, h = divmod(core, 2)
        t0 = h * H
        xb = np.zeros((T + L + 1, D), np.float32)
        xb[L + 1 :] = x[b]
        # window rows [t0 .. t0+W] in padded coords = tokens [t0-L-1 .. t0+H-1]
        win = xb[t0 : t0 + W1]                           # (W1, D)
        # partition-major: xin[p, g*W1 + t] = win[t, g*128 + p]
        xin = np.ascontiguousarray(
            win.T.reshape(G, P, W1).transpose(1, 0, 2).reshape(P, G * W1)
        ).astype(bf16)
        warm_bias = np.full(D, 0.0 if h == 1 else -10000.0, np.float32)
        params = np.stack([emw, eu, mk, mv, mr, warm_bias], axis=1)  # (D, NPAR)
        params = np.ascontiguousarray(
            params.reshape(G, P, NPAR).transpose(1, 0, 2).reshape(P, G * NPAR)
        ).astype(np.float32)
        in_maps.append({
            "xin": xin,
            "wk": wk_t, "wv": wv_t, "wr": wr_t, "wo": wo_t,
            "params": params,
        })
    return in_maps


def get_graph():
    global _built
    if _built is None:
        _built = build_graph()
    return _built


def kernel(**inputs) -> np.ndarray:
    from concourse.bass_utils import run_bass_kernel_spmd

    nc = get_graph()
    in_maps = prepare_inputs(**inputs)
    res = run_bass_kernel_spmd(nc, in_maps, list(range(N_CORES)))
    out = np.empty((B, T, D), np.float32)
    for core in range(N_CORES):
        b, h = divmod(core, 2)
        out[b, h * H : (h + 1) * H, :] = res.results[core]["out"].T
    return out
